# revision 1
# baseline (speedup 1.0000x reference)
"""Tensor-parallel attention layer (dense_transformer) for 8 Trainium2 cores.

Sharding: tensor-parallel over heads — each core owns H/8 = 2 heads:
its column slices of Wq/Wk/Wv, its KV-cache slice, and computes attention
for those heads. The output projection is sharded over token blocks after an
AllToAll that redistributes per-head context features into per-token blocks.

Layouts (per core):
  xT   [2048, 4096]  x transposed (feature-major), shared by all cores
  q/k  computed as qT/kT [256, 4096] (feature-major) via lhsT=W^T tiles
  v    computed token-major [4096, 256] via lhsT=xT tiles
  scores computed key-partition [u, s]; softmax denominator via ones-matmul
  on the PE; no max-subtraction (scores are O(1) by construction).

All matmul operands use float32r (full-rate PE, ~1e-4 rel err).
"""

import math
import os
import sys

if "/opt/trn_rl_repo" not in sys.path:
    sys.path.append("/opt/trn_rl_repo")

import numpy as np

import concourse.bass as bass
import concourse.mybir as mybir
from concourse.tile import TileContext
from concourse.bass_utils import run_bass_kernel_spmd

# ---------------------------------------------------------------- wait split
# This toolchain's walrus accepts only ONE sync wait per instruction; Tile
# attaches several. Split extras into standalone EventSemaphore instructions.
import orjson


def _split_waits_in_module(d: dict) -> dict:
    n = 0
    for func in d.get("functions", []):
        for block in func.get("blocks", []):
            insts = block.get("instructions")
            if not insts:
                continue
            out = []
            changed = False
            for inst in insts:
                si = inst.get("sync_info")
                waits = si.get("on_wait") if si else None
                if waits and len(waits) > 1:
                    splittable = [w for w in waits if not w.get("wait_reg")]
                    keep = [w for w in waits if w.get("wait_reg")]
                    if splittable:
                        keep.append(splittable[-1])
                        extras = splittable[:-1]
                    else:
                        extras = []
                    for w in extras:
                        n += 1
                        out.append({
                            "debug": inst.get("debug"),
                            "engine": inst["engine"],
                            "ins": [],
                            "name": f"{inst['name']}__sw{n}",
                            "opcode": "EventSemaphore",
                            "outs": [],
                            "sync_info": {"on_update": [], "on_wait": [w]},
                        })
                    si["on_wait"] = keep
                    changed = True
                out.append(inst)
            if changed:
                block["instructions"] = out
    return d


_wait_patch_applied = False


def _apply_wait_patch():
    global _wait_patch_applied
    if _wait_patch_applied:
        return
    _wait_patch_applied = True
    orig = bass.Bass.to_json_bytes

    def to_json_bytes(self) -> bytes:
        return orjson.dumps(_split_waits_in_module(orjson.loads(orig(self))))

    bass.Bass.to_json_bytes = to_json_bytes


# ---------------------------------------------------------------- constants
B, S, DIM = 4, 1024, 2048
H, D, T = 16, 128, 1024
NCORES = 8
HC = H // NCORES          # heads per core = 2
F = HC * D                # features per core = 256
NTOK = B * S              # 4096 flattened tokens
TB = NTOK // NCORES       # out-projection token block = 512
KT = DIM // 128           # 16 contraction tiles
NT = NTOK // 512          # 8 token n-tiles

F32 = mybir.dt.float32
F32R = mybir.dt.float32r

_prog_cache = None


def _build_program():
    _apply_wait_patch()
    nc = bass.Bass(num_devices=NCORES)

    xT = nc.dram_tensor("xT", [DIM, NTOK], F32R, kind="ExternalInput")
    wq = nc.dram_tensor("wq", [DIM, F], F32R, kind="ExternalInput")
    wk = nc.dram_tensor("wk", [DIM, F], F32R, kind="ExternalInput")
    wv = nc.dram_tensor("wv", [DIM, F], F32R, kind="ExternalInput")
    wo = nc.dram_tensor("wo", [DIM, DIM], F32R, kind="ExternalInput")
    kTc = nc.dram_tensor("kTc", [B, HC, D, T], F32R, kind="ExternalInput")
    vc = nc.dram_tensor("vc", [B, HC, T, D], F32R, kind="ExternalInput")
    masks = nc.dram_tensor("masks", [4, 128, 512], F32R, kind="ExternalInput")
    bq2 = nc.dram_tensor("bq2", [128, HC], F32, kind="ExternalInput")
    bk2 = nc.dram_tensor("bk2", [128, HC], F32, kind="ExternalInput")
    bvb = nc.dram_tensor("bvb", [128, F], F32R, kind="ExternalInput")
    bo16 = nc.dram_tensor("bo16", [128, KT], F32, kind="ExternalInput")
    ones_d = nc.dram_tensor("ones_d", [128, 1], F32R, kind="ExternalInput")
    ones_row_d = nc.dram_tensor("ones_row_d", [1, 128], F32R, kind="ExternalInput")

    kTn = nc.dram_tensor("kTn", [F, NTOK], F32, kind="ExternalOutput")
    vn = nc.dram_tensor("vn", [NTOK, F], F32, kind="ExternalOutput")
    outT_blk = nc.dram_tensor("outT_blk", [DIM, TB], F32, kind="ExternalOutput")

    scale = 1.0 / math.sqrt(D)

    with nc.allow_low_precision(reason="f32r attention pipeline"), \
         TileContext(nc) as tc:
        with (
            tc.tile_pool(name="consts", bufs=1) as cpool,
            tc.tile_pool(name="dram", bufs=1, space="DRAM") as dpool,
        ):
            mask_sb = cpool.tile([128, 4, 512], F32R, tag="mask")
            nc.sync.dma_start(out=mask_sb[:],
                              in_=masks[:].rearrange("o p f -> p o f"))
            ones = cpool.tile([128, 1], F32R, tag="ones")
            nc.sync.dma_start(out=ones[:], in_=ones_d[:])
            ones_row = cpool.tile([1, 128], F32R, tag="ones_row")
            nc.sync.dma_start(out=ones_row[:], in_=ones_row_d[:])
            bq_sb = cpool.tile([128, HC], F32, tag="bq")
            nc.sync.dma_start(out=bq_sb[:], in_=bq2[:])
            bk_sb = cpool.tile([128, HC], F32, tag="bk")
            nc.sync.dma_start(out=bk_sb[:], in_=bk2[:])
            bvb_sb = cpool.tile([128, F], F32R, tag="bvb")
            nc.sync.dma_start(out=bvb_sb[:], in_=bvb[:])
            bo_sb = cpool.tile([128, KT], F32, tag="bo")
            nc.sync.dma_start(out=bo_sb[:], in_=bo16[:])

            ag_in = dpool.tile([DIM, TB], F32)
            ag_out = dpool.tile([DIM, TB], F32)

            with tc.tile_pool(name="qkv", bufs=1) as qkvpool:
                qT_sb = [qkvpool.tile([128, NTOK], F32R, tag=f"qT{m}")
                         for m in range(HC)]
                kT_sb = [qkvpool.tile([128, NTOK], F32R, tag=f"kT{m}")
                         for m in range(HC)]
                v_sb = qkvpool.tile([128, NTOK // 128, F], F32R, tag="v")

                # ---------------- P1: QKV projections ----------------
                with (
                    tc.tile_pool(name="w1", bufs=1) as wpool,
                    tc.tile_pool(name="x1", bufs=2) as xpool,
                    tc.tile_pool(name="ps_qk", bufs=3, space="PSUM") as qkps,
                    tc.tile_pool(name="ps_v", bufs=3, space="PSUM") as vps,
                ):
                    wq_sb = wpool.tile([128, KT, F], F32R, tag="wq")
                    nc.sync.dma_start(
                        out=wq_sb[:],
                        in_=wq[:].rearrange("(k p) f -> p k f", p=128))
                    wk_sb = wpool.tile([128, KT, F], F32R, tag="wk")
                    nc.sync.dma_start(
                        out=wk_sb[:],
                        in_=wk[:].rearrange("(k p) f -> p k f", p=128))
                    wv_sb = wpool.tile([128, KT, F], F32R, tag="wv")
                    nc.sync.dma_start(
                        out=wv_sb[:],
                        in_=wv[:].rearrange("(k p) f -> p k f", p=128))

                    x_re = xT[:].rearrange("(k p) t -> p k t", p=128)
                    for n in range(NT):
                        tsl = slice(n * 512, (n + 1) * 512)
                        xn = xpool.tile([128, KT, 512], F32R, tag="xn")
                        nc.sync.dma_start(out=xn[:], in_=x_re[:, :, tsl])
                        for m in range(HC):
                            msl = slice(m * 128, (m + 1) * 128)
                            psq = qkps.tile([128, 512], F32, tag="psqk")
                            for k in range(KT):
                                nc.tensor.matmul(
                                    psq[:], wq_sb[:, k, msl], xn[:, k, :],
                                    start=(k == 0), stop=(k == KT - 1))
                            nc.vector.tensor_scalar_add(
                                out=qT_sb[m][:, tsl], in0=psq[:],
                                scalar1=bq_sb[:, m:m + 1])
                            psk = qkps.tile([128, 512], F32, tag="psqk")
                            for k in range(KT):
                                nc.tensor.matmul(
                                    psk[:], wk_sb[:, k, msl], xn[:, k, :],
                                    start=(k == 0), stop=(k == KT - 1))
                            nc.vector.tensor_scalar_add(
                                out=kT_sb[m][:, tsl], in0=psk[:],
                                scalar1=bk_sb[:, m:m + 1])
                            nc.sync.dma_start(
                                out=kTn[msl, tsl],
                                in_=kT_sb[m][:, tsl].bitcast(F32))
                        for mt in range(4):
                            tt = 4 * n + mt
                            psv = vps.tile([128, F], F32, tag="psv")
                            for k in range(KT):
                                nc.tensor.matmul(
                                    psv[:],
                                    xn[:, k, mt * 128:(mt + 1) * 128],
                                    wv_sb[:, k, :],
                                    start=(k == 0), stop=(k == KT - 1))
                            nc.vector.tensor_tensor(
                                out=v_sb[:, tt, :], in0=psv[:],
                                in1=bvb_sb[:], op=mybir.AluOpType.add)
                            nc.sync.dma_start(
                                out=vn[tt * 128:(tt + 1) * 128, :],
                                in_=v_sb[:, tt, :].bitcast(F32))

                # ---------------- P2: attention ----------------
                with (
                    tc.tile_pool(name="kc2", bufs=2) as kcpool,
                    tc.tile_pool(name="vc2", bufs=2) as vcpool,
                    tc.tile_pool(name="exp2", bufs=4) as expool,
                    tc.tile_pool(name="rec2", bufs=2) as recpool,
                    tc.tile_pool(name="cn2", bufs=2) as cnpool,
                    tc.tile_pool(name="ps_s", bufs=3, space="PSUM") as sps,
                    tc.tile_pool(name="ps_c", bufs=2, space="PSUM") as cps,
                    tc.tile_pool(name="ps_d", bufs=2, space="PSUM") as dps,
                    tc.tile_pool(name="ps_b", bufs=1, space="PSUM") as bps,
                ):
                    for b in range(B):
                        for h in range(HC):
                            kc = kcpool.tile([128, T], F32R, tag="kc")
                            nc.sync.dma_start(out=kc[:], in_=kTc[b, h])
                            vcs = vcpool.tile([128, T // 128, D], F32R,
                                              tag="vcs")
                            nc.sync.dma_start(
                                out=vcs[:],
                                in_=vc[b, h].rearrange("(u p) d -> p u d",
                                                       p=128))
                            for st in range(2):
                                s0 = 512 * st
                                q_rhs = qT_sb[h][:, b * S + s0:b * S + s0 + 512]
                                # (lhsT, v_ap, mask_idx)
                                usteps = []
                                for uk in range(T // 128):
                                    usteps.append((
                                        kc[:, uk * 128:(uk + 1) * 128],
                                        vcs[:, uk, :],
                                        None))
                                for j in range(4 * st):
                                    usteps.append((
                                        kT_sb[h][:, b * S + j * 128:
                                                 b * S + (j + 1) * 128],
                                        v_sb[:, b * 8 + j,
                                             h * 128:(h + 1) * 128],
                                        None))
                                for j in range(4 * st, 4 * st + 4):
                                    usteps.append((
                                        kT_sb[h][:, b * S + j * 128:
                                                 b * S + (j + 1) * 128],
                                        v_sb[:, b * 8 + j,
                                             h * 128:(h + 1) * 128],
                                        j - 4 * st))
                                U = len(usteps)
                                ps_c = cps.tile([128, 512], F32, tag="psc")
                                ps_d = dps.tile([1, 512], F32, tag="psd")
                                for ui, (k_lhs, v_ap, mo) in enumerate(usteps):
                                    ps_s = sps.tile([128, 512], F32,
                                                    tag="pss")
                                    nc.tensor.matmul(ps_s[:], k_lhs, q_rhs,
                                                     start=True, stop=True)
                                    ex = expool.tile([128, 512], F32R,
                                                     tag="ex")
                                    nc.scalar.activation(
                                        ex[:], ps_s[:],
                                        mybir.ActivationFunctionType.Exp,
                                        scale=scale)
                                    if mo is not None:
                                        nc.vector.tensor_tensor(
                                            out=ex[:], in0=ex[:],
                                            in1=mask_sb[:, mo, :],
                                            op=mybir.AluOpType.mult)
                                    nc.tensor.matmul(ps_c[:], v_ap, ex[:],
                                                     start=(ui == 0),
                                                     stop=(ui == U - 1))
                                    nc.tensor.matmul(ps_d[:], ones[:], ex[:],
                                                     start=(ui == 0),
                                                     stop=(ui == U - 1))
                                rec = recpool.tile([1, 512], F32R, tag="rec")
                                nc.vector.reciprocal(rec[:], ps_d[:])
                                recB = bps.tile([128, 512], F32, tag="recB")
                                nc.tensor.matmul(recB[:], ones_row[:], rec[:],
                                                 start=True, stop=True)
                                recB_sb = cnpool.tile([128, 512], F32,
                                                      tag="recB_sb")
                                nc.scalar.copy(recB_sb[:], recB[:])
                                cn = cnpool.tile([128, 512], F32, tag="cn")
                                nc.vector.tensor_tensor(
                                    out=cn[:], in0=ps_c[:], in1=recB_sb[:],
                                    op=mybir.AluOpType.mult)
                                # token block r = 2b + st, feature rows
                                # [128h, 128h+128) of this core's 256
                                r = 2 * b + st
                                nc.sync.dma_start(
                                    out=ag_in[r * F + h * 128:
                                              r * F + (h + 1) * 128, :],
                                    in_=cn[:])

            # ---------------- P3: AllToAll ----------------
            nc.gpsimd.collective_compute(
                "AllToAll",
                mybir.AluOpType.bypass,
                replica_groups=[list(range(NCORES))],
                ins=[ag_in.opt()],
                outs=[ag_out.opt()],
            )

            # ---------------- P4: output projection ----------------
            with (
                tc.tile_pool(name="cx4", bufs=1) as cxpool,
                tc.tile_pool(name="wo4", bufs=3) as wopool,
                tc.tile_pool(name="o4", bufs=3) as opool,
                tc.tile_pool(name="ps_o", bufs=4, space="PSUM") as ops,
            ):
                cx = cxpool.tile([128, KT, TB], F32, tag="cx")
                nc.sync.dma_start(
                    out=cx[:],
                    in_=ag_out[:].rearrange("(k p) t -> p k t", p=128))
                for m in range(KT):
                    wom = wopool.tile([128, KT, 128], F32R, tag="wom")
                    nc.sync.dma_start(
                        out=wom[:],
                        in_=wo[:, m * 128:(m + 1) * 128].rearrange(
                            "(k p) j -> p k j", p=128))
                    pso = ops.tile([128, TB], F32, tag="pso")
                    for k in range(KT):
                        nc.tensor.matmul(
                            pso[:], wom[:, k, :],
                            cx[:, k, :].bitcast(F32R),
                            start=(k == 0), stop=(k == KT - 1))
                    osb = opool.tile([128, TB], F32, tag="osb")
                    nc.vector.tensor_scalar_add(
                        out=osb[:], in0=pso[:], scalar1=bo_sb[:, m:m + 1])
                    nc.sync.dma_start(
                        out=outT_blk[m * 128:(m + 1) * 128, :], in_=osb[:])

    return nc


def _get_program():
    global _prog_cache
    if _prog_cache is None:
        _prog_cache = _build_program()
    return _prog_cache


def _make_masks() -> np.ndarray:
    p = np.arange(128)[:, None]
    f = np.arange(512)[None, :]
    return np.stack([(f - p - 128 * o >= 0).astype(np.float32)
                     for o in range(4)])


def kernel(x, k_cache, v_cache, Wq, bq, Wk, bk, Wv, bv, Wo, bo):
    x = np.asarray(x, dtype=np.float32)
    k_cache = np.asarray(k_cache, dtype=np.float32)
    v_cache = np.asarray(v_cache, dtype=np.float32)
    Wq = np.asarray(Wq, dtype=np.float32)
    Wk = np.asarray(Wk, dtype=np.float32)
    Wv = np.asarray(Wv, dtype=np.float32)
    Wo = np.asarray(Wo, dtype=np.float32)
    bq = np.asarray(bq, dtype=np.float32)
    bk = np.asarray(bk, dtype=np.float32)
    bv = np.asarray(bv, dtype=np.float32)
    bo = np.asarray(bo, dtype=np.float32)

    nc = _get_program()

    xT = np.ascontiguousarray(x.reshape(NTOK, DIM).T)
    woT = np.ascontiguousarray(Wo.T)
    masks = _make_masks()
    ones_col = np.ones((128, 1), np.float32)
    ones_row = np.ones((1, 128), np.float32)

    in_maps = []
    for c in range(NCORES):
        fs = slice(c * F, (c + 1) * F)
        hs = slice(c * HC, (c + 1) * HC)
        in_maps.append({
            "xT": xT,
            "wq": np.ascontiguousarray(Wq[fs].T),
            "wk": np.ascontiguousarray(Wk[fs].T),
            "wv": np.ascontiguousarray(Wv[fs].T),
            "wo": woT,
            "kTc": np.ascontiguousarray(k_cache[:, hs].transpose(0, 1, 3, 2)),
            "vc": np.ascontiguousarray(v_cache[:, hs]),
            "masks": masks,
            "bq2": np.ascontiguousarray(bq[fs].reshape(HC, 128).T),
            "bk2": np.ascontiguousarray(bk[fs].reshape(HC, 128).T),
            "bvb": np.broadcast_to(bv[fs], (128, F)).copy(),
            "bo16": np.ascontiguousarray(bo.reshape(KT, 128).T),
            "ones_d": ones_col,
            "ones_row_d": ones_row,
        })

    res = run_bass_kernel_spmd(
        nc, in_maps, list(range(NCORES)),
        trace=bool(int(os.environ.get("KERNEL_TRACE", "0"))))
    if res.exec_time_ns is not None:
        print(f"HW exec time: {res.exec_time_ns} ns")

    out = np.empty((NTOK, DIM), np.float32)
    k_out = np.empty((B, H, T + S, D), np.float32)
    v_out = np.empty((B, H, T + S, D), np.float32)
    k_out[:, :, :T] = k_cache
    v_out[:, :, :T] = v_cache
    for c in range(NCORES):
        r = res.results[c]
        out[c * TB:(c + 1) * TB, :] = r["outT_blk"].T
        hs = slice(c * HC, (c + 1) * HC)
        k_out[:, hs, T:] = r["kTn"].reshape(HC, 128, B, S).transpose(2, 0, 3, 1)
        v_out[:, hs, T:] = r["vn"].reshape(B, S, HC, D).transpose(0, 2, 1, 3)
    return (out.reshape(B, S, DIM), k_out, v_out)
